# revision 33
# baseline (speedup 1.0000x reference)
"""Trainium2 Bass kernel for nn_BasicBlock (binarized-conv ResNet block).

Reference computation (per-batch BN in training mode):
    out = BN1(x); out = binconv(sign(out), sign(w1-mean), g1*a1*b1); relu
    out = BN2(out); out = binconv(sign(out), sign(w2-mean), g2*a2*b2)
    out = BN3(out); out = relu(out + x)

Structure exploited:
  * BN1/BN2 outputs are consumed only through sign(), so each collapses to a
    per-channel affine threshold  sign(a*x + c)  with a = g*rsqrt(var+eps),
    c = b - mean*a  computed from *global* batch stats (sync-BN all-reduce).
  * Binarized activations/weights are exactly +-1 -> fp8e4m3 operands with
    fp32 PSUM accumulation are bit-exact.
  * Conv 3x3 pad 1 = 9 shifted matmuls accumulating in PSUM over a
    zero-padded SBUF image (30x30), contraction over input channels.

Sharding: data-parallel over batch. 64 images -> 8 cores x 8 images.
Three tiny ([128,4] fp32) AllReduces provide the sync-BN statistics.
"""

import sys

sys.path.insert(0, "/opt/trn_rl_repo")

import numpy as np
import ml_dtypes

import concourse.bass as bass
import concourse.tile as tile
import concourse.mybir as mybir
from concourse import bacc
from concourse.bass_utils import run_bass_kernel_spmd

F32 = mybir.dt.float32
FP8 = mybir.dt.float8e4
AF = mybir.ActivationFunctionType
ALU = mybir.AluOpType

N = 64
C = 256
P = 256
H = 28
W = 28
HW = H * W          # 784
HP = H + 2          # padded 30
WP = W + 2
HH = H // 2         # 14 rows per half-image
FREE = HH * W       # 392 = matmul free dim / PSUM tile (fits one 2KB bank)
# DoubleRow layout: padded image stored as 30 rows x 30 cols contiguous per
# chunk, chunk-pair stride padded to 912 (16B-aligned for the fp8 pair AP).
# Each matmul covers a contiguous 420-elem window (14 rows x 30 cols); the
# trailing 2 cols per row are over-compute that post-processing skips.
KO_STRIDE = 912     # 900 rows + 12 zero tail
DR_FREE = HH * HP   # 420
EPS = 1e-5

# columns of the packed per-channel parameter tensor (a1/c1 = host-side
# BN1 binarize coefficients: BN1 stats depend only on x, so they are
# computed on host like the weight binarization)
(COL_G1, COL_B1, COL_G2, COL_B2, COL_G3, COL_B3, COL_GA1, COL_GA2,
 COL_A1, COL_C1) = range(10)
NPARAM = 10


def _emit_conv(nc, w_tiles, xb_tiles, rs_big, acc_s, acc_q, params_sb,
               gamma_col, psum_pool, scratch_pool, nl, relu, dr, m_sb=None):
    """One 3x3 binary conv: 9 shifted matmuls per (out-chunk, c-chunk), PSUM
    accumulate, then scale by the precomputed gamma x (alpha x beta) map
    (m_sb, [128, 2, 2, HW], image-duplicated) with optional fused relu;
    writes rs_big[cko] ([128, nl*2, FREE], (image,half)-major).

    Each PSUM sub-tile gets one fused DVE pass rs = (psum max 0) * M whose
    accum_out also yields sum(rs); a single whole-block Pool stt pass
    squares rs into a throwaway while accumulating sum(rs^2) into acc_q.
    No bn_stats needed, and the DVE cadence stays under the PE block time
    so PE never stalls on PSUM banks.

    dr=True uses fp8 DoubleRow: the whole 256-channel contraction in one
    matmul pass per kernel position (xb tiles hold both channel chunks as
    [128, 2, HP*WP], weights as [128, 2, 128])."""
    assert dr
    n_blk = nl // 2
    # blk-major: each image-pair is processed for both channel chunks as
    # soon as its binarized tiles land, so conv1 tracks the x DMA stream
    # instead of waiting for the full batch before the cko=0 pass
    for blk in range(n_blk):
        for cko in range(2):
            pt = psum_pool.tile([128, 4, 512], F32, name="pt", tag="pt")
            for kh in range(3):
                for kw in range(3):
                    wt = w_tiles[cko][kh][kw]
                    first = kh == 0 and kw == 0
                    last = kh == 2 and kw == 2
                    for i2 in range(2):
                        xv = xb_tiles[blk * 2 + i2][:]
                        for half in range(2):
                            s = (half * HH + kh) * HP + kw
                            rhs = xv[:, :, s:s + DR_FREE]
                            nc.tensor.matmul(
                                pt[:][:, i2 * 2 + half, 0:DR_FREE], wt,
                                rhs, start=first, stop=last,
                                perf_mode=mybir.MatmulPerfMode.DoubleRow)
            # per-tile fused DVE pass (the compiler caps stt APs at 3D):
            # rs = max(psum,0) * (gamma.ab); sum(rs) -> acc_s column
            for q in range(4):
                half = q % 2
                pvq = (pt[:][:, q, 0:DR_FREE]
                       .rearrange("p (r w) -> p r w", w=HP)[:, :, 0:W])
                mvq = (m_sb[:][:, cko, half * FREE:(half + 1) * FREE]
                       .rearrange("p (r w) -> p r w", w=W))
                dvq = (rs_big[cko][:][:, blk * 4 + q]
                       .rearrange("p (r w) -> p r w", w=W))
                col = blk * 4 + q
                nc.vector.scalar_tensor_tensor(
                    dvq, pvq, 0.0, mvq,
                    op0=ALU.max if relu else ALU.add, op1=ALU.mult,
                    accum_out=acc_s[:][:, cko, col:col + 1])
            # one whole-block square pass; rs^2 into a throwaway with
            # sum(rs^2) -> acc_q. Alternate ACT/DVE so neither engine's
            # tail gates the sync-BN payload (Pool cannot accumulate).
            dst_flat = (rs_big[cko][:][:, blk * 4:(blk + 1) * 4]
                        .rearrange("p q f -> p (q f)"))
            dummy = scratch_pool.tile([128, 4 * FREE], F32, name="scr",
                                      tag="scr")
            if (blk * 2 + cko) % 2 == 0:
                nc.scalar.activation(dummy[:], dst_flat, AF.Square,
                                     accum_out=acc_q[:][:, cko, blk:blk + 1])
            else:
                nc.vector.scalar_tensor_tensor(
                    dummy[:], dst_flat, 0.0, dst_flat,
                    op0=ALU.add, op1=ALU.mult,
                    accum_out=acc_q[:][:, cko, blk:blk + 1])


def _conv_payload(nc, tmp_pool, acc_s, acc_q, n_cores, nl, tag):
    """Reduce the per-tile accum_out sums into the AllReduce payload
    (sum, sumsq)/(n_cores*nl*HW) -> summed over cores gives (global mean,
    global E[x^2])."""
    pay = tmp_pool.tile([128, 2, 2], F32, name=f"pay_{tag}", tag=f"pay_{tag}")
    for ck in range(2):
        nc.vector.tensor_reduce(pay[:][:, ck, 0:1], acc_s[:][:, ck],
                                axis=mybir.AxisListType.X, op=ALU.add)
        nc.vector.tensor_reduce(pay[:][:, ck, 1:2], acc_q[:][:, ck],
                                axis=mybir.AxisListType.X, op=ALU.add)
    nc.vector.tensor_scalar_mul(pay[:], pay[:], 1.0 / (n_cores * nl * HW))
    return pay[:]


def _stats_payload(nc, tmp_pool, st_tile, n_cores, tag):
    """bn_aggr the per-group 6-tuples, then build the AllReduce payload
    (mean/W, (var+mean^2)/W) -> sums to (global mean, global E[x^2])."""
    loc = tmp_pool.tile([128, 2, 2], F32, name=f"loc_{tag}", tag=f"loc_{tag}")
    for ck in range(2):
        nc.vector.bn_aggr(loc[:][:, ck],
                          st_tile[:][:, ck].rearrange("p a b c -> p (a b c)"))
    pay = tmp_pool.tile([128, 2, 2], F32, name=f"pay_{tag}", tag=f"pay_{tag}")
    msq = tmp_pool.tile([128, 2], F32, name=f"msq_{tag}", tag=f"msq_{tag}")
    nc.vector.tensor_mul(msq[:], loc[:][:, :, 0], loc[:][:, :, 0])
    nc.vector.tensor_add(pay[:][:, :, 1], loc[:][:, :, 1], msq[:])
    nc.vector.tensor_scalar_mul(pay[:][:, :, 1], pay[:][:, :, 1],
                                1.0 / n_cores)
    nc.vector.tensor_scalar_mul(pay[:][:, :, 0], loc[:][:, :, 0],
                                1.0 / n_cores)
    return pay[:]


def _emit_allreduce(nc, dram_pool, pay, g_sb, n_cores, tag,
                    use_collectives=True, bf=None, bn_idx=0, rep=0):
    """AllReduce(add) of pay [128,2,2] -> g_sb [128,2,2]. Payload columns are
    pre-divided so the sum over cores yields (global mean, global E[x^2]).

    bf is not None -> 3-step XOR-butterfly over remote_dma_broadcast
    (SBUF-to-SBUF, ~2-3us/step) instead of the ~20us ncfw AllReduce.
    Each step d in (1,2,4): send my partial to tpb^d's fixed slot, wait for
    the symmetric arrival (+2 on the step's remote sem), add. Slot buffers
    are reused across reps; safety comes from the three global sync points
    separating successive uses."""
    if not use_collectives:
        # cost-model build: skip the collective (n_cores==1 semantics)
        nc.vector.tensor_copy(g_sb[:], pay)
        return
    if bf is not None and bf.get("mode") == "ag":
        # AllGather (no 1.875x reduce phase) + local DVE tree-sum. The
        # gathered [8,128,4] comes back as [128,8,4] via a strided DMA.
        cin = dram_pool.tile([128, 4], F32, name=f"agi_{tag}",
                             tag=f"agi_{tag}")
        cout = dram_pool.tile([n_cores, 128, 4], F32, name=f"ago_{tag}",
                              tag=f"ago_{tag}")
        # payload DMAs ride the ACT hwdge queue so they are not stuck
        # behind the bulk x/weight stream on the sync queue
        nc.scalar.dma_start(cin[:], pay.rearrange("p a b -> p (a b)"))
        nc.gpsimd.collective_compute(
            "AllGather", ALU.bypass, replica_groups=[list(range(n_cores))],
            ins=[cin[:].opt()], outs=[cout[:].opt()])
        gall = bf["gall"][bn_idx]
        nc.scalar.dma_start(gall[:], cout[:].rearrange("r p f -> p r f"))
        gv = gall[:].rearrange("p r f -> p (r f)")
        t16 = bf["t16"][bn_idx]
        nc.vector.tensor_add(t16[:], gv[:, 0:16], gv[:, 16:32])
        t16v = t16[:]
        nc.vector.tensor_add(t16v[:, 0:8], t16v[:, 0:8], t16v[:, 8:16])
        nc.vector.tensor_add(g_sb[:].rearrange("p a b -> p (a b)"),
                             t16v[:, 0:4], t16v[:, 4:8])
        return
    if bf is not None:
        cur = pay.rearrange("p a b -> p (a b)")
        for si, d in enumerate((1, 2, 4)):
            slot = bf["slot"][bn_idx][si]
            rdests = [None] * 8
            rdests[d] = (0, d)
            nc.gpsimd.remote_dma_broadcast(
                slot[:], cur, remote_sem=bf["rsem"][si],
                local_sem=bf["lsem"], rdests=rdests)
            nc.gpsimd.trigger_dma(count=None)
            out_t = (g_sb.rearrange("p a b -> p (a b)") if si == 2
                     else bf["part"][bn_idx][si][:])
            add = nc.vector.tensor_add(out_t, cur, slot[:])
            # The peer-arrival wait cannot be simulated by Tile's single-core
            # scheduling pass (it would deadlock); inject it post-schedule.
            bf["deferred"].append(
                (add, bf["rsem"][si], 2 * (3 * rep + bn_idx + 1)))
            cur = out_t
        return
    cin = dram_pool.tile([128, 4], F32, name=f"ar_in_{tag}", tag=f"ar_in_{tag}")
    cout = dram_pool.tile([128, 4], F32, name=f"ar_out_{tag}", tag=f"ar_out_{tag}")
    nc.sync.dma_start(cin[:], pay.rearrange("p a b -> p (a b)"))
    nc.gpsimd.collective_compute(
        "AllReduce", ALU.add, replica_groups=[list(range(n_cores))],
        ins=[cin[:].opt()], outs=[cout[:].opt()])
    nc.sync.dma_start(g_sb[:].rearrange("p a b -> p (a b)"), cout[:])


def _emit_coeffs(nc, tmp_pool, g_sb, params_sb, gcol, bcol, a_sb, c_sb, tag):
    """a = g * rsqrt(var+eps), c = b - mean*a from g_sb=(mean, E[x^2])."""
    mean = g_sb[:][:, :, 0]
    e2 = g_sb[:][:, :, 1]
    var = tmp_pool.tile([128, 2], F32, name=f"var_{tag}", tag=f"var_{tag}")
    inv = tmp_pool.tile([128, 2], F32, name=f"inv_{tag}", tag=f"inv_{tag}")
    rsq = tmp_pool.tile([128, 2], F32, name=f"rsq_{tag}", tag=f"rsq_{tag}")
    nc.vector.tensor_mul(var[:], mean, mean)
    nc.vector.tensor_sub(var[:], e2, var[:])
    nc.vector.tensor_scalar_add(var[:], var[:], EPS)
    nc.vector.reciprocal(inv[:], var[:])
    nc.scalar.sqrt(rsq[:], inv[:])
    nc.vector.tensor_mul(a_sb[:], params_sb[:][:, :, gcol], rsq[:])
    nc.vector.tensor_mul(var[:], mean, a_sb[:])
    nc.vector.tensor_sub(c_sb[:], params_sb[:][:, :, bcol], var[:])


def _emit_binarize(nc, src_view, xb_tiles, a_sb, c_sb, nl):
    """xb[ck][n] interior = sign(a[ck]*src + c[ck]) as fp8 (+-1).

    Image-major order so the first conv block (images 0-1) unblocks after
    four ACT ops instead of waiting for a whole chunk."""
    for n in range(nl):
        for ck in range(2):
            nc.scalar.activation(
                xb_tiles[ck][n][:].rearrange("p (h w) -> p h w", h=HP)
                [:, 1:H + 1, 1:W + 1],
                src_view(ck, n),
                AF.Sign,
                bias=c_sb[:][:, ck:ck + 1],
                scale=a_sb[:][:, ck:ck + 1],
            )


def build_module(n_cores, nl, use_collectives=True, dr=True, reps=1,
                 ar_mode="cc", loop=1):
    """Build + schedule the SPMD module.

    dr:   fp8 DoubleRow matmuls (256-channel contraction per pass).
    loop: wrap the rep body in a hardware For_i executed `loop` times (for
          robust timing: device time scales with `loop` while NEFF size and
          dispatch overhead stay constant).
    reps: emit the whole computation `reps` times back-to-back in one NEFF
          (for wall-clock timing through the high-latency axon dispatch;
          device exec time ~= (wall(reps) - wall(1)) / (reps-1))."""
    nc = bacc.Bacc("TRN2", target_bir_lowering=False, debug=False,
                   enable_asserts=False, num_devices=n_cores)

    x_t = nc.dram_tensor("x", (nl, C, H, W), F32, kind="ExternalInput")
    wshape = (3, 3, 128, 2, P) if dr else (3, 3, C, P)
    wb1_t = nc.dram_tensor("wb1", wshape, FP8, kind="ExternalInput")
    wb2_t = nc.dram_tensor("wb2", wshape, FP8, kind="ExternalInput")
    params_t = nc.dram_tensor("params", (128, 2, NPARAM), F32, kind="ExternalInput")
    ab1_t = nc.dram_tensor("ab1", (128, HW), F32, kind="ExternalInput")
    ab2_t = nc.dram_tensor("ab2", (128, HW), F32, kind="ExternalInput")
    out_t = nc.dram_tensor("out", (nl, C, H, W), F32, kind="ExternalOutput")

    x_ap = x_t.ap()
    out_ap = out_t.ap()

    with tile.TileContext(nc) as tc:
        # ---------- pools ----------
        wp = tc.alloc_tile_pool(name="w", bufs=1)
        cp = tc.alloc_tile_pool(name="const", bufs=1)
        xbp = tc.alloc_tile_pool(name="xb", bufs=1)
        rsp = tc.alloc_tile_pool(name="rs", bufs=1)
        xap = tc.alloc_tile_pool(name="xa", bufs=1)
        stp = tc.alloc_tile_pool(name="st", bufs=1)
        tmp = tc.alloc_tile_pool(name="tmp", bufs=1)
        scratch = tc.alloc_tile_pool(name="scr", bufs=2)
        fin_pool = tc.alloc_tile_pool(name="fin", bufs=6)
        ob_pool = tc.alloc_tile_pool(name="ob", bufs=6)
        psum_pool = tc.alloc_tile_pool(name="ps", bufs=2, space="PSUM")
        dram_pool = tc.alloc_tile_pool(name="drm", bufs=1, space="DRAM")

        # ---------- one-time: constants (small, first) + zeroed pads ------
        # x loads are emitted FIRST in phase A; the weight loads (below, as a
        # deferred closure) are emitted after them so the serial DMA stream
        # drains x early and BN1's AllReduce isn't pushed out by weights.
        ab1_sb = cp.tile([128, HW], F32, name="ab1", tag="ab1")
        ab2_sb = cp.tile([128, HW], F32, name="ab2", tag="ab2")
        params_sb = cp.tile([128, 2, NPARAM], F32, name="params", tag="params")
        # params first: binarize1 needs its a1/c1 columns immediately,
        # while the ab maps are only consumed by the M-map build later
        nc.sync.dma_start(params_sb[:], params_t.ap())
        nc.sync.dma_start(ab1_sb[:], ab1_t.ap())
        nc.sync.dma_start(ab2_sb[:], ab2_t.ap())

        w_tiles = [None, None]

        def emit_weight_loads():
            # one big DMA per conv layer (1152 512B descriptors) instead of
            # 36 small ones: ~1.7us vs ~9us on the serial DMA stream
            for ci, wap in enumerate([wb1_t.ap(), wb2_t.ap()]):
                big = wp.tile([128, 9, 2, P], FP8, name=f"wb{ci}",
                              tag=f"wb{ci}")
                nc.sync.dma_start(
                    big[:], wap.rearrange("kh kw p i o -> p (kh kw) i o"))
                w_tiles[ci] = [
                    [[big[:][:, kh * 3 + kw, :,
                             cko * 128:(cko + 1) * 128]
                      for kw in range(3)] for kh in range(3)]
                    for cko in range(2)]

        # gamma (x) alpha.beta maps: one fused DVE pass per post-conv tile
        m1_sb = cp.tile([128, 2, HW], F32, name="m1", tag="m1")
        m2_sb = cp.tile([128, 2, HW], F32, name="m2", tag="m2")

        def emit_m_maps():
            for cko in range(2):
                nc.vector.tensor_scalar_mul(
                    m1_sb[:][:, cko], ab1_sb[:],
                    params_sb[:][:, cko, COL_GA1:COL_GA1 + 1])
                nc.vector.tensor_scalar_mul(
                    m2_sb[:][:, cko], ab2_sb[:],
                    params_sb[:][:, cko, COL_GA2:COL_GA2 + 1])

        # padded binarized activations (fp8, zero halo; borders stay zero
        # across reps because only interiors are ever rewritten)
        if dr:
            xb1 = [xbp.tile([128, 2, KO_STRIDE], FP8, name=f"xb1_{n}",
                            tag=f"xb1_{n}") for n in range(nl)]
            xb2 = [xbp.tile([128, 2, KO_STRIDE], FP8, name=f"xb2_{n}",
                            tag=f"xb2_{n}") for n in range(nl)]
            for t in xb1 + xb2:
                nc.gpsimd.memzero(t[:])

            def xb_interior(xb, ck, n):
                return (xb[n][:][:, ck, 0:HP * WP]
                        .rearrange("p (h w) -> p h w", h=HP)
                        [:, 1:H + 1, 1:W + 1])
        else:
            xb1 = [[xbp.tile([128, HP * WP], FP8, name=f"xb1_{ck}_{n}",
                             tag=f"xb1_{ck}_{n}") for n in range(nl)]
                   for ck in range(2)]
            xb2 = [[xbp.tile([128, HP * WP], FP8, name=f"xb2_{ck}_{n}",
                             tag=f"xb2_{ck}_{n}") for n in range(nl)]
                   for ck in range(2)]
            for row in xb1 + xb2:
                for t in row:
                    nc.gpsimd.memzero(t[:])

            def xb_interior(xb, ck, n):
                return (xb[ck][n][:].rearrange("p (h w) -> p h w", h=HP)
                        [:, 1:H + 1, 1:W + 1])

        # r1 / s2 storage (aliased: s2 overwrites r1 once consumed) and
        # resident x (used for BN1 stats, binarize, and the final residual)
        rs = [rsp.tile([128, nl * 2, FREE], F32, name=f"rs_{ck}",
                       tag=f"rs_{ck}") for ck in range(2)]

        def rs_img(ck, n):
            return rs[ck][:][:, 2 * n:2 * n + 2].rearrange("p a b -> p (a b)")
        xa = {}
        for ck in range(2):
            for n in range(nl):
                xa[ck, n] = xap.tile([128, HW], F32, name=f"xa_{ck}_{n}",
                                     tag=f"xa_{ck}_{n}")

        nb = nl // 2
        acc_s1 = stp.tile([128, 2, nl * 2], F32, name="acc_s1", tag="acc_s1")
        acc_q1 = stp.tile([128, 2, nb], F32, name="acc_q1", tag="acc_q1")
        acc_s2 = stp.tile([128, 2, nl * 2], F32, name="acc_s2", tag="acc_s2")
        acc_q2 = stp.tile([128, 2, nb], F32, name="acc_q2", tag="acc_q2")

        g2_sb = tmp.tile([128, 2, 2], F32, name="g2", tag="g2")
        g3_sb = tmp.tile([128, 2, 2], F32, name="g3", tag="g3")
        a2_sb = tmp.tile([128, 2], F32, name="a2", tag="a2")
        c2_sb = tmp.tile([128, 2], F32, name="c2", tag="c2")
        a3_sb = tmp.tile([128, 2], F32, name="a3", tag="a3")
        c3_sb = tmp.tile([128, 2], F32, name="c3", tag="c3")

        bf = None
        if use_collectives and ar_mode == "rdma" and n_cores == 8:
            bf = {
                "deferred": [],
                "rsem": [nc.alloc_semaphore(f"bf_rsem{k}") for k in range(3)],
                "lsem": nc.alloc_semaphore("bf_lsem"),
                "slot": [[stp.tile([128, 4], F32, name=f"bfs_{b}_{s}",
                                   tag=f"bfs_{b}_{s}") for s in range(3)]
                         for b in range(3)],
                "part": [[tmp.tile([128, 4], F32, name=f"bfp_{b}_{s}",
                                   tag=f"bfp_{b}_{s}") for s in range(2)]
                         for b in range(3)],
            }
            nc.has_collectives = True
        elif use_collectives and ar_mode == "ag":
            bf = {
                "mode": "ag",
                "gall": [tmp.tile([128, n_cores, 4], F32, name=f"gall{b}",
                                  tag=f"gall{b}") for b in range(3)],
                "t16": [tmp.tile([128, 16], F32, name=f"t16_{b}",
                                 tag=f"t16_{b}") for b in range(3)],
            }

        def binarize(src_view, xb, a_ap, c_ap):
            # image-major so the first conv block unblocks earliest
            for n in range(nl):
                for ck in range(2):
                    nc.scalar.activation(
                        xb_interior(xb, ck, n), src_view(ck, n), AF.Sign,
                        bias=c_ap[:, ck:ck + 1], scale=a_ap[:, ck:ck + 1])

        import contextlib
        if loop > 1:
            # hoist the one-time loads out of the hardware loop
            emit_weight_loads()
            emit_m_maps()
        loop_cm = (tc.For_i(0, loop) if loop > 1
                   else contextlib.nullcontext())
        with loop_cm:
          a1_ap = params_sb[:][:, :, COL_A1]
        c1_ap = params_sb[:][:, :, COL_C1]

        for rep in range(reps):
            # ---------- phase A: load x; binarize1 with host-side BN1
            # coefficients as each tile lands. Weight loads are emitted
            # behind the first two images so conv1 can start while the
            # rest of x streams in.
            for n in range(nl):
                for ck in range(2):
                    t = xa[ck, n]
                    nc.sync.dma_start(
                        t[:].rearrange("p (h w) -> p h w", h=H),
                        x_ap[n, ck * 128:(ck + 1) * 128])
                    nc.scalar.activation(
                        xb_interior(xb1, ck, n),
                        t[:].rearrange("p (h w) -> p h w", h=H), AF.Sign,
                        bias=c1_ap[:, ck:ck + 1], scale=a1_ap[:, ck:ck + 1])
                if n == 1 and rep == 0 and loop == 1:
                    emit_weight_loads()
            if rep == 0 and loop == 1:
                emit_m_maps()

            # ---------- conv1 (+ relu) ----------
            _emit_conv(nc, w_tiles[0], xb1, rs, acc_s1, acc_q1, params_sb,
                       COL_GA1, psum_pool, scratch, nl, relu=True, dr=dr,
                       m_sb=m1_sb)

            pay2 = _conv_payload(nc, tmp, acc_s1, acc_q1, n_cores, nl, "bn2")
            _emit_allreduce(nc, dram_pool, pay2, g2_sb, n_cores, "bn2",
                            use_collectives, bf, 1, rep)
            _emit_coeffs(nc, tmp, g2_sb, params_sb, COL_G2, COL_B2,
                         a2_sb, c2_sb, "bn2")
            binarize(lambda ck, n: rs_img(ck, n)
                     .rearrange("p (h w) -> p h w", h=H), xb2,
                     a2_sb[:], c2_sb[:])

            # ---------- conv2 (no relu); s2 overwrites rs ----------
            _emit_conv(nc, w_tiles[1], xb2, rs, acc_s2, acc_q2, params_sb,
                       COL_GA2, psum_pool, scratch, nl, relu=False, dr=dr,
                       m_sb=m2_sb)

            pay3 = _conv_payload(nc, tmp, acc_s2, acc_q2, n_cores, nl, "bn3")
            _emit_allreduce(nc, dram_pool, pay3, g3_sb, n_cores, "bn3",
                            use_collectives, bf, 2, rep)
            _emit_coeffs(nc, tmp, g3_sb, params_sb, COL_G3, COL_B3,
                         a3_sb, c3_sb, "bn3")

            # ---------- final: out = relu(a3*s2 + c3 + x) ----------
            # engine-split: the a3*s2+x pass alternates DVE/Pool, the
            # relu(.+c3) pass alternates ACT/DVE, so no single engine
            # serializes the 16-tile tail behind AR3.
            for i, (ck, n) in enumerate([(ck, n) for ck in range(2)
                                         for n in range(nl)]):
                a3_ap = a3_sb[:][:, ck:ck + 1]
                c3_ap = c3_sb[:][:, ck:ck + 1]
                s2v = rs_img(ck, n)
                t1 = fin_pool.tile([128, HW], F32, name="fin", tag="fin")
                nc.vector.scalar_tensor_tensor(
                    t1[:], s2v, a3_ap, xa[ck, n][:],
                    op0=ALU.mult, op1=ALU.add)
                ob = ob_pool.tile([128, HW], F32, name="ob", tag="ob")
                nc.scalar.activation(ob[:], t1[:], AF.Relu, bias=c3_ap)
                nc.sync.dma_start(
                    out_ap[n, ck * 128:(ck + 1) * 128],
                    ob[:].rearrange("p (h w) -> p h w", h=H))

        for pool in (dram_pool, psum_pool, ob_pool, fin_pool, scratch, tmp,
                     stp, xap, rsp, xbp, cp, wp):
            pool.release()

    if bf is not None and "deferred" in bf:
        # The scheduled adds' wait slots are full, so splice a standalone
        # EventSemaphore wait onto the same engine right before each one.
        import bass_rust as _br
        targets = {inst.ins.name: (sem, val)
                   for (inst, sem, val) in bf["deferred"]}
        for blk in nc.m.functions[0].blocks:
            new_insts = []
            for ins_ in blk.instructions:
                if ins_.name in targets:
                    sem, val = targets.pop(ins_.name)
                    ev = mybir.InstEventSemaphore(
                        name=f"bfwait_{ins_.name}", ins=[], outs=[])
                    ev.engine = ins_.engine
                    _br.wait_op(ev, sem, val, "sem-ge", True)
                    new_insts.append(ev)
                new_insts.append(ins_)
            blk.instructions[:] = new_insts
        assert not targets, f"unmatched butterfly waits: {targets}"
    nc.compile()
    return nc


def host_inputs(x, bn1_g, bn1_b, bn2_g, bn2_b, bn3_g, bn3_b,
                w1, gamma1, alpha1, beta1, w2, gamma2, alpha2, beta2,
                dr=True):
    """Host-side prep: binarize weights, pack per-channel params, alpha x beta
    outer-product maps."""
    fp8 = ml_dtypes.float8_e4m3

    def binw(w):
        centered = w - np.mean(w, axis=1, keepdims=True, dtype=np.float32)
        wb = np.sign(centered).astype(np.float32)
        # (P, C, 3, 3) -> (3, 3, C, P)
        wb = np.ascontiguousarray(wb.transpose(2, 3, 1, 0))
        if dr:
            # DoubleRow interleave: c = ko*128 + ki -> (3, 3, ki, ko, P)
            wb = np.ascontiguousarray(
                wb.reshape(3, 3, 2, 128, P).transpose(0, 1, 3, 2, 4))
        return wb.astype(fp8)

    wb1 = binw(w1)
    wb2 = binw(w2)

    xf = np.asarray(x, dtype=np.float32)
    m1 = xf.mean(axis=(0, 2, 3), dtype=np.float64)
    v1 = (xf.astype(np.float64) ** 2).mean(axis=(0, 2, 3)) - m1 ** 2
    a1 = (np.asarray(bn1_g, np.float64)
          / np.sqrt(v1 + EPS)).astype(np.float32)
    c1 = (np.asarray(bn1_b, np.float32)
          - m1.astype(np.float32) * a1).astype(np.float32)
    cols = [bn1_g, bn1_b, bn2_g, bn2_b, bn3_g, bn3_b, gamma1, gamma2, a1, c1]
    params = np.stack([np.asarray(c, np.float32) for c in cols], axis=-1)
    params = np.ascontiguousarray(
        params.reshape(2, 128, NPARAM).transpose(1, 0, 2))  # (128, 2, NPARAM)

    ab1 = np.ascontiguousarray(
        np.broadcast_to(np.outer(alpha1, beta1).reshape(-1), (128, HW))
    ).astype(np.float32)
    ab2 = np.ascontiguousarray(
        np.broadcast_to(np.outer(alpha2, beta2).reshape(-1), (128, HW))
    ).astype(np.float32)
    return wb1, wb2, params, ab1, ab2


_MODULE_CACHE = {}


def get_module(n_cores, nl, use_collectives=True, dr=True, reps=1,
               ar_mode="cc", loop=1):
    key = (n_cores, nl, use_collectives, dr, reps, ar_mode, loop)
    if key not in _MODULE_CACHE:
        _MODULE_CACHE[key] = build_module(n_cores, nl, use_collectives,
                                          dr=dr, reps=reps, ar_mode=ar_mode,
                                          loop=loop)
    return _MODULE_CACHE[key]


def kernel(x, bn1_g, bn1_b, bn2_g, bn2_b, bn3_g, bn3_b,
           w1, gamma1, alpha1, beta1, w2, gamma2, alpha2, beta2,
           _trace=False):
    n_cores = 8
    dr = True
    nl = x.shape[0] // n_cores
    nc = get_module(n_cores, nl, dr=dr, ar_mode="ag")

    wb1, wb2, params, ab1, ab2 = host_inputs(
        x, bn1_g, bn1_b, bn2_g, bn2_b, bn3_g, bn3_b,
        w1, gamma1, alpha1, beta1, w2, gamma2, alpha2, beta2, dr=dr)

    x = np.ascontiguousarray(np.asarray(x, dtype=np.float32))
    in_maps = []
    for i in range(n_cores):
        in_maps.append({
            "x": np.ascontiguousarray(x[i * nl:(i + 1) * nl]),
            "wb1": wb1, "wb2": wb2, "params": params,
            "ab1": ab1, "ab2": ab2,
        })

    res = run_bass_kernel_spmd(nc, in_maps, core_ids=list(range(n_cores)),
                               trace=_trace)
    out = np.concatenate([r["out"] for r in res.results], axis=0)
    kernel.last_results = res
    return out


if __name__ == "__main__":
    np.random.seed(0)
    xs = np.random.randn(64, C, H, W).astype(np.float32)
    print("module build only")
    get_module(8, 8)
    print("built ok")



# revision 35
# speedup vs baseline: 1.9394x; 1.9394x over previous
"""Trainium2 Bass kernel for nn_BasicBlock (binarized-conv ResNet block).

Reference computation (per-batch BN in training mode):
    out = BN1(x); out = binconv(sign(out), sign(w1-mean), g1*a1*b1); relu
    out = BN2(out); out = binconv(sign(out), sign(w2-mean), g2*a2*b2)
    out = BN3(out); out = relu(out + x)

Structure exploited:
  * BN1/BN2 outputs are consumed only through sign(), so each collapses to a
    per-channel affine threshold  sign(a*x + c)  with a = g*rsqrt(var+eps),
    c = b - mean*a  computed from *global* batch stats (sync-BN all-reduce).
  * Binarized activations/weights are exactly +-1 -> fp8e4m3 operands with
    fp32 PSUM accumulation are bit-exact.
  * Conv 3x3 pad 1 = 9 shifted matmuls accumulating in PSUM over a
    zero-padded SBUF image (30x30), contraction over input channels.

Sharding: data-parallel over batch. 64 images -> 8 cores x 8 images.
Three tiny ([128,4] fp32) AllReduces provide the sync-BN statistics.
"""

import sys

sys.path.insert(0, "/opt/trn_rl_repo")

import numpy as np
import ml_dtypes

import concourse.bass as bass
import concourse.tile as tile
import concourse.mybir as mybir
from concourse import bacc
from concourse.bass_utils import run_bass_kernel_spmd

F32 = mybir.dt.float32
FP8 = mybir.dt.float8e4
AF = mybir.ActivationFunctionType
ALU = mybir.AluOpType

N = 64
C = 256
P = 256
H = 28
W = 28
HW = H * W          # 784
HP = H + 2          # padded 30
WP = W + 2
HH = H // 2         # 14 rows per half-image
FREE = HH * W       # 392 = matmul free dim / PSUM tile (fits one 2KB bank)
# DoubleRow layout: padded image stored as 30 rows x 30 cols contiguous per
# chunk, chunk-pair stride padded to 912 (16B-aligned for the fp8 pair AP).
# Each matmul covers a contiguous 420-elem window (14 rows x 30 cols); the
# trailing 2 cols per row are over-compute that post-processing skips.
KO_STRIDE = 912     # 900 rows + 12 zero tail
DR_FREE = HH * HP   # 420
EPS = 1e-5

# columns of the packed per-channel parameter tensor (a1/c1 = host-side
# BN1 binarize coefficients: BN1 stats depend only on x, so they are
# computed on host like the weight binarization)
(COL_G1, COL_B1, COL_G2, COL_B2, COL_G3, COL_B3, COL_GA1, COL_GA2,
 COL_A1, COL_C1) = range(10)
NPARAM = 10


def _emit_conv(nc, w_tiles, xb_tiles, rs_big, acc_s, acc_q, params_sb,
               gamma_col, psum_pool, scratch_pool, nl, relu, dr, m_sb=None):
    """One 3x3 binary conv: 9 shifted matmuls per (out-chunk, c-chunk), PSUM
    accumulate, then scale by the precomputed gamma x (alpha x beta) map
    (m_sb, [128, 2, 2, HW], image-duplicated) with optional fused relu;
    writes rs_big[cko] ([128, nl*2, FREE], (image,half)-major).

    Each PSUM sub-tile gets one fused DVE pass rs = (psum max 0) * M whose
    accum_out also yields sum(rs); a single whole-block Pool stt pass
    squares rs into a throwaway while accumulating sum(rs^2) into acc_q.
    No bn_stats needed, and the DVE cadence stays under the PE block time
    so PE never stalls on PSUM banks.

    dr=True uses fp8 DoubleRow: the whole 256-channel contraction in one
    matmul pass per kernel position (xb tiles hold both channel chunks as
    [128, 2, HP*WP], weights as [128, 2, 128])."""
    assert dr
    n_blk = nl // 2
    # blk-major: each image-pair is processed for both channel chunks as
    # soon as its binarized tiles land, so conv1 tracks the x DMA stream
    # instead of waiting for the full batch before the cko=0 pass
    for blk in range(n_blk):
        for cko in range(2):
            pt = psum_pool.tile([128, 4, 512], F32, name="pt", tag="pt")
            for kh in range(3):
                for kw in range(3):
                    wt = w_tiles[cko][kh][kw]
                    first = kh == 0 and kw == 0
                    last = kh == 2 and kw == 2
                    for i2 in range(2):
                        xv = xb_tiles[blk * 2 + i2][:]
                        for half in range(2):
                            s = (half * HH + kh) * HP + kw
                            rhs = xv[:, :, s:s + DR_FREE]
                            nc.tensor.matmul(
                                pt[:][:, i2 * 2 + half, 0:DR_FREE], wt,
                                rhs, start=first, stop=last,
                                perf_mode=mybir.MatmulPerfMode.DoubleRow)
            # per-tile fused DVE pass (the compiler caps stt APs at 3D):
            # rs = max(psum,0) * (gamma.ab); sum(rs) -> acc_s column
            for q in range(4):
                half = q % 2
                pvq = (pt[:][:, q, 0:DR_FREE]
                       .rearrange("p (r w) -> p r w", w=HP)[:, :, 0:W])
                mvq = (m_sb[:][:, cko, half * FREE:(half + 1) * FREE]
                       .rearrange("p (r w) -> p r w", w=W))
                dvq = (rs_big[cko][:][:, blk * 4 + q]
                       .rearrange("p (r w) -> p r w", w=W))
                col = blk * 4 + q
                nc.vector.scalar_tensor_tensor(
                    dvq, pvq, 0.0, mvq,
                    op0=ALU.max if relu else ALU.add, op1=ALU.mult,
                    accum_out=acc_s[:][:, cko, col:col + 1])
            # one whole-block square pass; rs^2 into a throwaway with
            # sum(rs^2) -> acc_q. Alternate ACT/DVE so neither engine's
            # tail gates the sync-BN payload (Pool cannot accumulate).
            dst_flat = (rs_big[cko][:][:, blk * 4:(blk + 1) * 4]
                        .rearrange("p q f -> p (q f)"))
            dummy = scratch_pool.tile([128, 4 * FREE], F32, name="scr",
                                      tag="scr")
            if (blk * 2 + cko) % 2 == 0:
                nc.scalar.activation(dummy[:], dst_flat, AF.Square,
                                     accum_out=acc_q[:][:, cko, blk:blk + 1])
            else:
                nc.vector.scalar_tensor_tensor(
                    dummy[:], dst_flat, 0.0, dst_flat,
                    op0=ALU.add, op1=ALU.mult,
                    accum_out=acc_q[:][:, cko, blk:blk + 1])


def _conv_payload(nc, tmp_pool, acc_s, acc_q, n_cores, nl, tag):
    """Reduce the per-tile accum_out sums into the AllReduce payload
    (sum, sumsq)/(n_cores*nl*HW) -> summed over cores gives (global mean,
    global E[x^2])."""
    pay = tmp_pool.tile([128, 2, 2], F32, name=f"pay_{tag}", tag=f"pay_{tag}")
    for ck in range(2):
        nc.vector.tensor_reduce(pay[:][:, ck, 0:1], acc_s[:][:, ck],
                                axis=mybir.AxisListType.X, op=ALU.add)
        nc.vector.tensor_reduce(pay[:][:, ck, 1:2], acc_q[:][:, ck],
                                axis=mybir.AxisListType.X, op=ALU.add)
    nc.vector.tensor_scalar_mul(pay[:], pay[:], 1.0 / (n_cores * nl * HW))
    return pay[:]


def _stats_payload(nc, tmp_pool, st_tile, n_cores, tag):
    """bn_aggr the per-group 6-tuples, then build the AllReduce payload
    (mean/W, (var+mean^2)/W) -> sums to (global mean, global E[x^2])."""
    loc = tmp_pool.tile([128, 2, 2], F32, name=f"loc_{tag}", tag=f"loc_{tag}")
    for ck in range(2):
        nc.vector.bn_aggr(loc[:][:, ck],
                          st_tile[:][:, ck].rearrange("p a b c -> p (a b c)"))
    pay = tmp_pool.tile([128, 2, 2], F32, name=f"pay_{tag}", tag=f"pay_{tag}")
    msq = tmp_pool.tile([128, 2], F32, name=f"msq_{tag}", tag=f"msq_{tag}")
    nc.vector.tensor_mul(msq[:], loc[:][:, :, 0], loc[:][:, :, 0])
    nc.vector.tensor_add(pay[:][:, :, 1], loc[:][:, :, 1], msq[:])
    nc.vector.tensor_scalar_mul(pay[:][:, :, 1], pay[:][:, :, 1],
                                1.0 / n_cores)
    nc.vector.tensor_scalar_mul(pay[:][:, :, 0], loc[:][:, :, 0],
                                1.0 / n_cores)
    return pay[:]


def _emit_allreduce(nc, dram_pool, pay, g_sb, n_cores, tag,
                    use_collectives=True, bf=None, bn_idx=0, rep=0):
    """AllReduce(add) of pay [128,2,2] -> g_sb [128,2,2]. Payload columns are
    pre-divided so the sum over cores yields (global mean, global E[x^2]).

    bf is not None -> 3-step XOR-butterfly over remote_dma_broadcast
    (SBUF-to-SBUF, ~2-3us/step) instead of the ~20us ncfw AllReduce.
    Each step d in (1,2,4): send my partial to tpb^d's fixed slot, wait for
    the symmetric arrival (+2 on the step's remote sem), add. Slot buffers
    are reused across reps; safety comes from the three global sync points
    separating successive uses."""
    if not use_collectives:
        # cost-model build: skip the collective (n_cores==1 semantics)
        nc.vector.tensor_copy(g_sb[:], pay)
        return
    if bf is not None and bf.get("mode") == "ag":
        # AllGather (no 1.875x reduce phase) + local DVE tree-sum. The
        # gathered [8,128,4] comes back as [128,8,4] via a strided DMA.
        cin = dram_pool.tile([128, 4], F32, name=f"agi_{tag}",
                             tag=f"agi_{tag}")
        cout = dram_pool.tile([n_cores, 128, 4], F32, name=f"ago_{tag}",
                              tag=f"ago_{tag}")
        # payload DMAs ride the ACT hwdge queue so they are not stuck
        # behind the bulk x/weight stream on the sync queue
        nc.scalar.dma_start(cin[:], pay.rearrange("p a b -> p (a b)"))
        nc.gpsimd.collective_compute(
            "AllGather", ALU.bypass, replica_groups=[list(range(n_cores))],
            ins=[cin[:].opt()], outs=[cout[:].opt()])
        gall = bf["gall"][bn_idx]
        nc.scalar.dma_start(gall[:], cout[:].rearrange("r p f -> p r f"))
        gv = gall[:].rearrange("p r f -> p (r f)")
        t16 = bf["t16"][bn_idx]
        nc.vector.tensor_add(t16[:], gv[:, 0:16], gv[:, 16:32])
        t16v = t16[:]
        nc.vector.tensor_add(t16v[:, 0:8], t16v[:, 0:8], t16v[:, 8:16])
        nc.vector.tensor_add(g_sb[:].rearrange("p a b -> p (a b)"),
                             t16v[:, 0:4], t16v[:, 4:8])
        return
    if bf is not None:
        cur = pay.rearrange("p a b -> p (a b)")
        for si, d in enumerate((1, 2, 4)):
            slot = bf["slot"][bn_idx][si]
            rdests = [None] * 8
            rdests[d] = (0, d)
            nc.gpsimd.remote_dma_broadcast(
                slot[:], cur, remote_sem=bf["rsem"][si],
                local_sem=bf["lsem"], rdests=rdests)
            nc.gpsimd.trigger_dma(count=None)
            out_t = (g_sb.rearrange("p a b -> p (a b)") if si == 2
                     else bf["part"][bn_idx][si][:])
            add = nc.vector.tensor_add(out_t, cur, slot[:])
            # The peer-arrival wait cannot be simulated by Tile's single-core
            # scheduling pass (it would deadlock); inject it post-schedule.
            bf["deferred"].append(
                (add, bf["rsem"][si], 2 * (3 * rep + bn_idx + 1)))
            cur = out_t
        return
    cin = dram_pool.tile([128, 4], F32, name=f"ar_in_{tag}", tag=f"ar_in_{tag}")
    cout = dram_pool.tile([128, 4], F32, name=f"ar_out_{tag}", tag=f"ar_out_{tag}")
    nc.sync.dma_start(cin[:], pay.rearrange("p a b -> p (a b)"))
    nc.gpsimd.collective_compute(
        "AllReduce", ALU.add, replica_groups=[list(range(n_cores))],
        ins=[cin[:].opt()], outs=[cout[:].opt()])
    nc.sync.dma_start(g_sb[:].rearrange("p a b -> p (a b)"), cout[:])


def _emit_coeffs(nc, tmp_pool, g_sb, params_sb, gcol, bcol, a_sb, c_sb, tag):
    """a = g * rsqrt(var+eps), c = b - mean*a from g_sb=(mean, E[x^2])."""
    mean = g_sb[:][:, :, 0]
    e2 = g_sb[:][:, :, 1]
    var = tmp_pool.tile([128, 2], F32, name=f"var_{tag}", tag=f"var_{tag}")
    inv = tmp_pool.tile([128, 2], F32, name=f"inv_{tag}", tag=f"inv_{tag}")
    rsq = tmp_pool.tile([128, 2], F32, name=f"rsq_{tag}", tag=f"rsq_{tag}")
    nc.vector.tensor_mul(var[:], mean, mean)
    nc.vector.tensor_sub(var[:], e2, var[:])
    nc.vector.tensor_scalar_add(var[:], var[:], EPS)
    nc.vector.reciprocal(inv[:], var[:])
    nc.scalar.sqrt(rsq[:], inv[:])
    nc.vector.tensor_mul(a_sb[:], params_sb[:][:, :, gcol], rsq[:])
    nc.vector.tensor_mul(var[:], mean, a_sb[:])
    nc.vector.tensor_sub(c_sb[:], params_sb[:][:, :, bcol], var[:])


def _emit_binarize(nc, src_view, xb_tiles, a_sb, c_sb, nl):
    """xb[ck][n] interior = sign(a[ck]*src + c[ck]) as fp8 (+-1).

    Image-major order so the first conv block (images 0-1) unblocks after
    four ACT ops instead of waiting for a whole chunk."""
    for n in range(nl):
        for ck in range(2):
            nc.scalar.activation(
                xb_tiles[ck][n][:].rearrange("p (h w) -> p h w", h=HP)
                [:, 1:H + 1, 1:W + 1],
                src_view(ck, n),
                AF.Sign,
                bias=c_sb[:][:, ck:ck + 1],
                scale=a_sb[:][:, ck:ck + 1],
            )


def build_module(n_cores, nl, use_collectives=True, dr=True, reps=1,
                 ar_mode="cc", loop=1):
    """Build + schedule the SPMD module.

    dr:   fp8 DoubleRow matmuls (256-channel contraction per pass).
    loop: wrap the rep body in a hardware For_i executed `loop` times (for
          robust timing: device time scales with `loop` while NEFF size and
          dispatch overhead stay constant).
    reps: emit the whole computation `reps` times back-to-back in one NEFF
          (for wall-clock timing through the high-latency axon dispatch;
          device exec time ~= (wall(reps) - wall(1)) / (reps-1))."""
    nc = bacc.Bacc("TRN2", target_bir_lowering=False, debug=False,
                   enable_asserts=False, num_devices=n_cores)

    x_t = nc.dram_tensor("x", (nl, C, H, W), F32, kind="ExternalInput")
    wshape = (3, 3, 128, 2, P) if dr else (3, 3, C, P)
    wb1_t = nc.dram_tensor("wb1", wshape, FP8, kind="ExternalInput")
    wb2_t = nc.dram_tensor("wb2", wshape, FP8, kind="ExternalInput")
    params_t = nc.dram_tensor("params", (128, 2, NPARAM), F32, kind="ExternalInput")
    ab1_t = nc.dram_tensor("ab1", (128, HW), F32, kind="ExternalInput")
    ab2_t = nc.dram_tensor("ab2", (128, HW), F32, kind="ExternalInput")
    out_t = nc.dram_tensor("out", (nl, C, H, W), F32, kind="ExternalOutput")

    x_ap = x_t.ap()
    out_ap = out_t.ap()

    with tile.TileContext(nc) as tc:
        # ---------- pools ----------
        wp = tc.alloc_tile_pool(name="w", bufs=1)
        cp = tc.alloc_tile_pool(name="const", bufs=1)
        xbp = tc.alloc_tile_pool(name="xb", bufs=1)
        rsp = tc.alloc_tile_pool(name="rs", bufs=1)
        xap = tc.alloc_tile_pool(name="xa", bufs=1)
        stp = tc.alloc_tile_pool(name="st", bufs=1)
        tmp = tc.alloc_tile_pool(name="tmp", bufs=1)
        scratch = tc.alloc_tile_pool(name="scr", bufs=2)
        fin_pool = tc.alloc_tile_pool(name="fin", bufs=6)
        ob_pool = tc.alloc_tile_pool(name="ob", bufs=6)
        psum_pool = tc.alloc_tile_pool(name="ps", bufs=2, space="PSUM")
        dram_pool = tc.alloc_tile_pool(name="drm", bufs=1, space="DRAM")

        # ---------- one-time: constants (small, first) + zeroed pads ------
        # x loads are emitted FIRST in phase A; the weight loads (below, as a
        # deferred closure) are emitted after them so the serial DMA stream
        # drains x early and BN1's AllReduce isn't pushed out by weights.
        ab1_sb = cp.tile([128, HW], F32, name="ab1", tag="ab1")
        ab2_sb = cp.tile([128, HW], F32, name="ab2", tag="ab2")
        params_sb = cp.tile([128, 2, NPARAM], F32, name="params", tag="params")
        # params first: binarize1 needs its a1/c1 columns immediately,
        # while the ab maps are only consumed by the M-map build later
        nc.sync.dma_start(params_sb[:], params_t.ap())
        nc.sync.dma_start(ab1_sb[:], ab1_t.ap())
        nc.sync.dma_start(ab2_sb[:], ab2_t.ap())

        w_tiles = [None, None]

        def emit_weight_loads():
            # one big DMA per conv layer (1152 512B descriptors) instead of
            # 36 small ones: ~1.7us vs ~9us on the serial DMA stream
            for ci, wap in enumerate([wb1_t.ap(), wb2_t.ap()]):
                big = wp.tile([128, 9, 2, P], FP8, name=f"wb{ci}",
                              tag=f"wb{ci}")
                nc.sync.dma_start(
                    big[:], wap.rearrange("kh kw p i o -> p (kh kw) i o"))
                w_tiles[ci] = [
                    [[big[:][:, kh * 3 + kw, :,
                             cko * 128:(cko + 1) * 128]
                      for kw in range(3)] for kh in range(3)]
                    for cko in range(2)]

        # gamma (x) alpha.beta maps: one fused DVE pass per post-conv tile
        m1_sb = cp.tile([128, 2, HW], F32, name="m1", tag="m1")
        m2_sb = cp.tile([128, 2, HW], F32, name="m2", tag="m2")

        def emit_m_maps():
            for cko in range(2):
                nc.vector.tensor_scalar_mul(
                    m1_sb[:][:, cko], ab1_sb[:],
                    params_sb[:][:, cko, COL_GA1:COL_GA1 + 1])
                nc.vector.tensor_scalar_mul(
                    m2_sb[:][:, cko], ab2_sb[:],
                    params_sb[:][:, cko, COL_GA2:COL_GA2 + 1])

        # padded binarized activations (fp8, zero halo; borders stay zero
        # across reps because only interiors are ever rewritten)
        if dr:
            xb1 = [xbp.tile([128, 2, KO_STRIDE], FP8, name=f"xb1_{n}",
                            tag=f"xb1_{n}") for n in range(nl)]
            xb2 = [xbp.tile([128, 2, KO_STRIDE], FP8, name=f"xb2_{n}",
                            tag=f"xb2_{n}") for n in range(nl)]
            for t in xb1 + xb2:
                nc.gpsimd.memzero(t[:])

            def xb_interior(xb, ck, n):
                return (xb[n][:][:, ck, 0:HP * WP]
                        .rearrange("p (h w) -> p h w", h=HP)
                        [:, 1:H + 1, 1:W + 1])
        else:
            xb1 = [[xbp.tile([128, HP * WP], FP8, name=f"xb1_{ck}_{n}",
                             tag=f"xb1_{ck}_{n}") for n in range(nl)]
                   for ck in range(2)]
            xb2 = [[xbp.tile([128, HP * WP], FP8, name=f"xb2_{ck}_{n}",
                             tag=f"xb2_{ck}_{n}") for n in range(nl)]
                   for ck in range(2)]
            for row in xb1 + xb2:
                for t in row:
                    nc.gpsimd.memzero(t[:])

            def xb_interior(xb, ck, n):
                return (xb[ck][n][:].rearrange("p (h w) -> p h w", h=HP)
                        [:, 1:H + 1, 1:W + 1])

        # r1 / s2 storage (aliased: s2 overwrites r1 once consumed) and
        # resident x (used for BN1 stats, binarize, and the final residual)
        rs = [rsp.tile([128, nl * 2, FREE], F32, name=f"rs_{ck}",
                       tag=f"rs_{ck}") for ck in range(2)]

        def rs_img(ck, n):
            return rs[ck][:][:, 2 * n:2 * n + 2].rearrange("p a b -> p (a b)")
        xa = {}
        for ck in range(2):
            for n in range(nl):
                xa[ck, n] = xap.tile([128, HW], F32, name=f"xa_{ck}_{n}",
                                     tag=f"xa_{ck}_{n}")

        nb = nl // 2
        acc_s1 = stp.tile([128, 2, nl * 2], F32, name="acc_s1", tag="acc_s1")
        acc_q1 = stp.tile([128, 2, nb], F32, name="acc_q1", tag="acc_q1")
        acc_s2 = stp.tile([128, 2, nl * 2], F32, name="acc_s2", tag="acc_s2")
        acc_q2 = stp.tile([128, 2, nb], F32, name="acc_q2", tag="acc_q2")

        g2_sb = tmp.tile([128, 2, 2], F32, name="g2", tag="g2")
        g3_sb = tmp.tile([128, 2, 2], F32, name="g3", tag="g3")
        a2_sb = tmp.tile([128, 2], F32, name="a2", tag="a2")
        c2_sb = tmp.tile([128, 2], F32, name="c2", tag="c2")
        a3_sb = tmp.tile([128, 2], F32, name="a3", tag="a3")
        c3_sb = tmp.tile([128, 2], F32, name="c3", tag="c3")

        bf = None
        if use_collectives and ar_mode == "rdma" and n_cores == 8:
            bf = {
                "deferred": [],
                "rsem": [nc.alloc_semaphore(f"bf_rsem{k}") for k in range(3)],
                "lsem": nc.alloc_semaphore("bf_lsem"),
                "slot": [[stp.tile([128, 4], F32, name=f"bfs_{b}_{s}",
                                   tag=f"bfs_{b}_{s}") for s in range(3)]
                         for b in range(3)],
                "part": [[tmp.tile([128, 4], F32, name=f"bfp_{b}_{s}",
                                   tag=f"bfp_{b}_{s}") for s in range(2)]
                         for b in range(3)],
            }
            nc.has_collectives = True
        elif use_collectives and ar_mode == "ag":
            bf = {
                "mode": "ag",
                "gall": [tmp.tile([128, n_cores, 4], F32, name=f"gall{b}",
                                  tag=f"gall{b}") for b in range(3)],
                "t16": [tmp.tile([128, 16], F32, name=f"t16_{b}",
                                 tag=f"t16_{b}") for b in range(3)],
            }

        def binarize(src_view, xb, a_ap, c_ap):
            # image-major so the first conv block unblocks earliest
            for n in range(nl):
                for ck in range(2):
                    nc.scalar.activation(
                        xb_interior(xb, ck, n), src_view(ck, n), AF.Sign,
                        bias=c_ap[:, ck:ck + 1], scale=a_ap[:, ck:ck + 1])

        import contextlib
        if loop > 1:
            # hoist the one-time loads out of the hardware loop
            emit_weight_loads()
            emit_m_maps()
        loop_cm = (tc.For_i(0, loop) if loop > 1
                   else contextlib.nullcontext())
        with loop_cm:
          a1_ap = params_sb[:][:, :, COL_A1]
        c1_ap = params_sb[:][:, :, COL_C1]

        for rep in range(reps):
            # ---------- phase A: load x; binarize1 with host-side BN1
            # coefficients as each tile lands. Weight loads are emitted
            # behind the first two images so conv1 can start while the
            # rest of x streams in.
            for n in range(nl):
                for ck in range(2):
                    t = xa[ck, n]
                    nc.sync.dma_start(
                        t[:].rearrange("p (h w) -> p h w", h=H),
                        x_ap[n, ck * 128:(ck + 1) * 128])
                    nc.scalar.activation(
                        xb_interior(xb1, ck, n),
                        t[:].rearrange("p (h w) -> p h w", h=H), AF.Sign,
                        bias=c1_ap[:, ck:ck + 1], scale=a1_ap[:, ck:ck + 1])
                if n == 1 and rep == 0 and loop == 1:
                    emit_weight_loads()
            if rep == 0 and loop == 1:
                emit_m_maps()

            # ---------- conv1 (+ relu) ----------
            _emit_conv(nc, w_tiles[0], xb1, rs, acc_s1, acc_q1, params_sb,
                       COL_GA1, psum_pool, scratch, nl, relu=True, dr=dr,
                       m_sb=m1_sb)

            pay2 = _conv_payload(nc, tmp, acc_s1, acc_q1, n_cores, nl, "bn2")
            _emit_allreduce(nc, dram_pool, pay2, g2_sb, n_cores, "bn2",
                            use_collectives, bf, 1, rep)
            _emit_coeffs(nc, tmp, g2_sb, params_sb, COL_G2, COL_B2,
                         a2_sb, c2_sb, "bn2")
            binarize(lambda ck, n: rs_img(ck, n)
                     .rearrange("p (h w) -> p h w", h=H), xb2,
                     a2_sb[:], c2_sb[:])

            # ---------- conv2 (no relu); s2 overwrites rs ----------
            _emit_conv(nc, w_tiles[1], xb2, rs, acc_s2, acc_q2, params_sb,
                       COL_GA2, psum_pool, scratch, nl, relu=False, dr=dr,
                       m_sb=m2_sb)

            pay3 = _conv_payload(nc, tmp, acc_s2, acc_q2, n_cores, nl, "bn3")
            _emit_allreduce(nc, dram_pool, pay3, g3_sb, n_cores, "bn3",
                            use_collectives, bf, 2, rep)
            _emit_coeffs(nc, tmp, g3_sb, params_sb, COL_G3, COL_B3,
                         a3_sb, c3_sb, "bn3")

            # ---------- final: out = relu(a3*s2 + c3 + x) ----------
            # engine-split: the a3*s2+x pass alternates DVE/Pool, the
            # relu(.+c3) pass alternates ACT/DVE, so no single engine
            # serializes the 16-tile tail behind AR3.
            for i, (ck, n) in enumerate([(ck, n) for ck in range(2)
                                         for n in range(nl)]):
                a3_ap = a3_sb[:][:, ck:ck + 1]
                c3_ap = c3_sb[:][:, ck:ck + 1]
                s2v = rs_img(ck, n)
                t1 = fin_pool.tile([128, HW], F32, name="fin", tag="fin")
                nc.vector.scalar_tensor_tensor(
                    t1[:], s2v, a3_ap, xa[ck, n][:],
                    op0=ALU.mult, op1=ALU.add)
                ob = ob_pool.tile([128, HW], F32, name="ob", tag="ob")
                nc.scalar.activation(ob[:], t1[:], AF.Relu, bias=c3_ap)
                nc.sync.dma_start(
                    out_ap[n, ck * 128:(ck + 1) * 128],
                    ob[:].rearrange("p (h w) -> p h w", h=H))

        for pool in (dram_pool, psum_pool, ob_pool, fin_pool, scratch, tmp,
                     stp, xap, rsp, xbp, cp, wp):
            pool.release()

    if bf is not None and "deferred" in bf:
        # The scheduled adds' wait slots are full, so splice a standalone
        # EventSemaphore wait onto the same engine right before each one.
        import bass_rust as _br
        targets = {inst.ins.name: (sem, val)
                   for (inst, sem, val) in bf["deferred"]}
        for blk in nc.m.functions[0].blocks:
            new_insts = []
            for ins_ in blk.instructions:
                if ins_.name in targets:
                    sem, val = targets.pop(ins_.name)
                    ev = mybir.InstEventSemaphore(
                        name=f"bfwait_{ins_.name}", ins=[], outs=[])
                    ev.engine = ins_.engine
                    _br.wait_op(ev, sem, val, "sem-ge", True)
                    new_insts.append(ev)
                new_insts.append(ins_)
            blk.instructions[:] = new_insts
        assert not targets, f"unmatched butterfly waits: {targets}"
    nc.compile()
    return nc


def host_inputs(x, bn1_g, bn1_b, bn2_g, bn2_b, bn3_g, bn3_b,
                w1, gamma1, alpha1, beta1, w2, gamma2, alpha2, beta2,
                dr=True):
    """Host-side prep: binarize weights, pack per-channel params, alpha x beta
    outer-product maps."""
    fp8 = ml_dtypes.float8_e4m3

    def binw(w):
        centered = w - np.mean(w, axis=1, keepdims=True, dtype=np.float32)
        wb = np.sign(centered).astype(np.float32)
        # (P, C, 3, 3) -> (3, 3, C, P)
        wb = np.ascontiguousarray(wb.transpose(2, 3, 1, 0))
        if dr:
            # DoubleRow interleave: c = ko*128 + ki -> (3, 3, ki, ko, P)
            wb = np.ascontiguousarray(
                wb.reshape(3, 3, 2, 128, P).transpose(0, 1, 3, 2, 4))
        return wb.astype(fp8)

    wb1 = binw(w1)
    wb2 = binw(w2)

    xf = np.asarray(x, dtype=np.float32)
    m1 = xf.mean(axis=(0, 2, 3), dtype=np.float64)
    v1 = (xf.astype(np.float64) ** 2).mean(axis=(0, 2, 3)) - m1 ** 2
    a1 = (np.asarray(bn1_g, np.float64)
          / np.sqrt(v1 + EPS)).astype(np.float32)
    c1 = (np.asarray(bn1_b, np.float32)
          - m1.astype(np.float32) * a1).astype(np.float32)
    cols = [bn1_g, bn1_b, bn2_g, bn2_b, bn3_g, bn3_b, gamma1, gamma2, a1, c1]
    params = np.stack([np.asarray(c, np.float32) for c in cols], axis=-1)
    params = np.ascontiguousarray(
        params.reshape(2, 128, NPARAM).transpose(1, 0, 2))  # (128, 2, NPARAM)

    ab1 = np.ascontiguousarray(
        np.broadcast_to(np.outer(alpha1, beta1).reshape(-1), (128, HW))
    ).astype(np.float32)
    ab2 = np.ascontiguousarray(
        np.broadcast_to(np.outer(alpha2, beta2).reshape(-1), (128, HW))
    ).astype(np.float32)
    return wb1, wb2, params, ab1, ab2


_MODULE_CACHE = {}


def get_module(n_cores, nl, use_collectives=True, dr=True, reps=1,
               ar_mode="cc", loop=1):
    key = (n_cores, nl, use_collectives, dr, reps, ar_mode, loop)
    if key not in _MODULE_CACHE:
        _MODULE_CACHE[key] = build_module(n_cores, nl, use_collectives,
                                          dr=dr, reps=reps, ar_mode=ar_mode,
                                          loop=loop)
    return _MODULE_CACHE[key]


def kernel(x, bn1_g, bn1_b, bn2_g, bn2_b, bn3_g, bn3_b,
           w1, gamma1, alpha1, beta1, w2, gamma2, alpha2, beta2,
           _trace=False):
    n_cores = 8
    dr = True
    nl = x.shape[0] // n_cores
    nc = get_module(n_cores, nl, dr=dr, ar_mode="ag")

    wb1, wb2, params, ab1, ab2 = host_inputs(
        x, bn1_g, bn1_b, bn2_g, bn2_b, bn3_g, bn3_b,
        w1, gamma1, alpha1, beta1, w2, gamma2, alpha2, beta2, dr=dr)

    x = np.ascontiguousarray(np.asarray(x, dtype=np.float32))
    in_maps = []
    for i in range(n_cores):
        in_maps.append({
            "x": np.ascontiguousarray(x[i * nl:(i + 1) * nl]),
            "wb1": wb1, "wb2": wb2, "params": params,
            "ab1": ab1, "ab2": ab2,
        })

    res = run_bass_kernel_spmd(nc, in_maps, core_ids=list(range(n_cores)),
                               trace=_trace)
    out = np.concatenate([r["out"] for r in res.results], axis=0)
    kernel.last_results = res
    return out


if __name__ == "__main__":
    np.random.seed(0)
    xs = np.random.randn(64, C, H, W).astype(np.float32)
    print("module build only")
    get_module(8, 8)
    print("built ok")

